# revision 7
# baseline (speedup 1.0000x reference)
"""Trainium2 Bass kernel for nn_BatchedTeacherPolicy.

2048 independent per-teacher MLPs (obs-norm -> 48->512->256->128->12,
ELU between layers, tanh at the end). Pure data parallel: 256 teachers
per NeuronCore across 8 cores.

Layout: teacher-on-partition. Each SBUF partition holds one teacher's
weights/activations; the per-teacher matvec y[o] = b[o] + sum_i W[o,i]x[i]
is one fused DVE tensor_tensor_reduce per output neuron o, computed for
128 teachers (partitions) simultaneously. Weight DMAs are fully
contiguous per partition (W[n, o0:o1, :] blocks).
"""

from contextlib import ExitStack

import numpy as np

import concourse.bass as bass
import concourse.bacc as bacc
import concourse.tile as tile
from concourse import mybir
from concourse.bass_utils import run_bass_kernel_spmd

N, OBS = 2048, 48
DIMS = [(512, 48), (256, 512), (128, 256), (12, 128)]  # (out, in) per layer
N_CORES = 8
NPC = N // N_CORES  # teachers per core
P = 128             # partitions = teachers per group
G = NPC // P        # groups per core
# o-chunk per layer: sized so W DMA chunks are ~2-4 MB
OCHUNK = [128, 16, 32, 12]

F32 = mybir.dt.float32
AF = mybir.ActivationFunctionType
ALU = mybir.AluOpType

_cached = {}


def _build_bass():
    nc = bacc.Bacc(trn_type="TRN2", target_bir_lowering=False)

    obs_d = nc.dram_tensor("obs", [NPC, OBS], F32, kind="ExternalInput")
    mean_d = nc.dram_tensor("mean", [NPC, OBS], F32, kind="ExternalInput")
    std_d = nc.dram_tensor("std", [NPC, OBS], F32, kind="ExternalInput")
    W_d, b_d = [], []
    for li, (o, i) in enumerate(DIMS):
        W_d.append(nc.dram_tensor(f"W{li}", [NPC, o, i], F32, kind="ExternalInput"))
        b_d.append(nc.dram_tensor(f"b{li}", [NPC, o], F32, kind="ExternalInput"))
    out_d = nc.dram_tensor("out", [NPC, DIMS[-1][0]], F32, kind="ExternalOutput")

    with ExitStack() as ctx:
        tc = ctx.enter_context(tile.TileContext(nc))
        wpool = ctx.enter_context(tc.tile_pool(name="wpool", bufs=3))
        xpool = ctx.enter_context(tc.tile_pool(name="xpool", bufs=3))
        spool = ctx.enter_context(tc.tile_pool(name="spool", bufs=2))
        bpool = ctx.enter_context(tc.tile_pool(name="bpool", bufs=2))

        for g in range(G):
            n0 = g * P

            # ---- obs normalization: x0 = clip((obs - mean)/std, -5, 5) ----
            obs_t = spool.tile([P, OBS], F32, tag="nrm")
            nc.sync.dma_start(out=obs_t, in_=obs_d[n0 : n0 + P, :])
            mean_t = spool.tile([P, OBS], F32, tag="nrm")
            nc.sync.dma_start(out=mean_t, in_=mean_d[n0 : n0 + P, :])
            std_t = spool.tile([P, OBS], F32, tag="nrm")
            nc.sync.dma_start(out=std_t, in_=std_d[n0 : n0 + P, :])

            # Each DVE op may carry at most ONE new semaphore wait (TRN2
            # TT-struct limit), so feed multi-operand ops through
            # single-input ops that absorb the DMA waits first.
            nmean = spool.tile([P, OBS], F32, tag="nmean")
            nc.vector.tensor_scalar_mul(nmean, mean_t, -1.0)
            rstd = spool.tile([P, OBS], F32, tag="rstd")
            nc.vector.reciprocal(rstd, std_t)
            x = xpool.tile([P, OBS], F32, tag="x", name=f"x_in_{g}")
            nc.vector.tensor_add(x, obs_t, nmean)
            nc.vector.tensor_mul(x, x, rstd)
            nc.vector.tensor_scalar(
                out=x, in0=x, scalar1=-5.0, scalar2=5.0,
                op0=ALU.max, op1=ALU.min,
            )

            # ---- MLP layers ----
            for li, (O, I) in enumerate(DIMS):
                bt = bpool.tile([P, O], F32, tag="bias", name=f"b_{g}_{li}")
                nc.sync.dma_start(out=bt, in_=b_d[li][n0 : n0 + P, :])
                y = xpool.tile([P, O], F32, tag="y", name=f"y_{g}_{li}")
                for c0 in range(0, O, OCHUNK[li]):
                    oc = min(OCHUNK[li], O - c0)
                    wt = wpool.tile([P, oc, I], F32, tag="w", name=f"w_{g}_{li}_{c0}")
                    nc.sync.dma_start(
                        out=wt, in_=W_d[li][n0 : n0 + P, c0 : c0 + oc, :]
                    )
                    scr = spool.tile([P, I], F32, tag="scr", name=f"scr_{g}_{li}_{c0}")
                    for o in range(oc):
                        # accum_out = sum_i W[o,i]*x[i]  (custom DVE fused
                        # multiply-reduce; the ISA TENSOR_TENSOR_REDUCE
                        # opcode crashes TRN2 hardware on this path)
                        nc.vector.affine_mul_reduce(
                            out=scr,
                            accum_out=y[:, c0 + o : c0 + o + 1],
                            in0=wt[:, o, :],
                            in1=x,
                            scale=1.0,
                            bias=0.0,
                        )
                nc.vector.tensor_add(y, y, bt)
                if li < len(DIMS) - 1:
                    # ELU(y) = exp(min(y,0)) + max(y,0) - 1
                    e = spool.tile([P, O], F32, tag="elu", name=f"e_{g}_{li}")
                    nc.vector.tensor_scalar_min(e, y, 0.0)
                    nc.scalar.activation(e, e, AF.Exp)
                    xn = xpool.tile([P, O], F32, tag="x", name=f"x_{g}_{li}")
                    nc.vector.scalar_tensor_tensor(
                        out=xn, in0=y, scalar=0.0, in1=e,
                        op0=ALU.max, op1=ALU.add,
                    )
                    nc.vector.tensor_scalar_add(xn, xn, -1.0)
                    x = xn
                else:
                    yt = xpool.tile([P, O], F32, tag="yt", name=f"yt_{g}")
                    nc.scalar.activation(yt, y, AF.Tanh)
                    nc.sync.dma_start(out=out_d[n0 : n0 + P, :], in_=yt)

    nc.compile()
    return nc


def _get_nc():
    if "nc" not in _cached:
        _cached["nc"] = _build_bass()
    return _cached["nc"]


def kernel(obs, mean, std, W0, b0, W1, b1, W2, b2, W3, b3, _trace=False):
    nc = _get_nc()
    full = {
        "obs": obs, "mean": mean, "std": std,
        "W0": W0, "b0": b0, "W1": W1, "b1": b1,
        "W2": W2, "b2": b2, "W3": W3, "b3": b3,
    }
    in_maps = []
    for c in range(N_CORES):
        sl = slice(c * NPC, (c + 1) * NPC)
        in_maps.append(
            {k: np.ascontiguousarray(np.asarray(v)[sl]) for k, v in full.items()}
        )
    res = run_bass_kernel_spmd(
        nc, in_maps, core_ids=list(range(N_CORES)), trace=_trace
    )
    _cached["last_results"] = res
    out = np.concatenate([res.results[c]["out"] for c in range(N_CORES)], axis=0)
    return out


# revision 9
# speedup vs baseline: 1.1320x; 1.1320x over previous
"""Trainium2 Bass kernel for nn_BatchedTeacherPolicy.

2048 independent per-teacher MLPs (obs-norm -> 48->512->256->128->12,
ELU between layers, tanh at the end). Pure data parallel: 256 teachers
per NeuronCore across 8 cores.

Layout: teacher-on-partition. Each SBUF partition holds one teacher's
weights/activations; the per-teacher matvec y[o] = b[o] + sum_i W[o,i]x[i]
is one fused DVE tensor_tensor_reduce per output neuron o, computed for
128 teachers (partitions) simultaneously. Weight DMAs are fully
contiguous per partition (W[n, o0:o1, :] blocks).
"""

from contextlib import ExitStack

import numpy as np

import concourse.bass as bass
import concourse.bacc as bacc
import concourse.tile as tile
from concourse import mybir
from concourse.bass_utils import run_bass_kernel_spmd

N, OBS = 2048, 48
DIMS = [(512, 48), (256, 512), (128, 256), (12, 128)]  # (out, in) per layer
N_CORES = 8
NPC = N // N_CORES  # teachers per core
P = 128             # partitions = teachers per group
G = NPC // P        # groups per core
# o-chunk per layer: sized so W DMA chunks are ~2-4 MB
OCHUNK = [128, 16, 32, 12]

F32 = mybir.dt.float32
AF = mybir.ActivationFunctionType
ALU = mybir.AluOpType

_cached = {}


def _build_bass():
    nc = bacc.Bacc(trn_type="TRN2", target_bir_lowering=False)

    obs_d = nc.dram_tensor("obs", [NPC, OBS], F32, kind="ExternalInput")
    mean_d = nc.dram_tensor("mean", [NPC, OBS], F32, kind="ExternalInput")
    std_d = nc.dram_tensor("std", [NPC, OBS], F32, kind="ExternalInput")
    W_d, b_d = [], []
    for li, (o, i) in enumerate(DIMS):
        W_d.append(nc.dram_tensor(f"W{li}", [NPC, o, i], F32, kind="ExternalInput"))
        b_d.append(nc.dram_tensor(f"b{li}", [NPC, o], F32, kind="ExternalInput"))
    out_d = nc.dram_tensor("out", [NPC, DIMS[-1][0]], F32, kind="ExternalOutput")

    with ExitStack() as ctx:
        tc = ctx.enter_context(tile.TileContext(nc))
        wpool = ctx.enter_context(tc.tile_pool(name="wpool", bufs=4))
        xpool = ctx.enter_context(tc.tile_pool(name="xpool", bufs=3))
        spool = ctx.enter_context(tc.tile_pool(name="spool", bufs=2))
        bpool = ctx.enter_context(tc.tile_pool(name="bpool", bufs=2))

        for g in range(G):
            n0 = g * P

            # ---- obs normalization: x0 = clip((obs - mean)/std, -5, 5) ----
            obs_t = spool.tile([P, OBS], F32, tag="nrm")
            nc.sync.dma_start(out=obs_t, in_=obs_d[n0 : n0 + P, :])
            mean_t = spool.tile([P, OBS], F32, tag="nrm")
            nc.sync.dma_start(out=mean_t, in_=mean_d[n0 : n0 + P, :])
            std_t = spool.tile([P, OBS], F32, tag="nrm")
            nc.sync.dma_start(out=std_t, in_=std_d[n0 : n0 + P, :])

            # Each DVE op may carry at most ONE new semaphore wait (TRN2
            # TT-struct limit), so feed multi-operand ops through
            # single-input ops that absorb the DMA waits first.
            nmean = spool.tile([P, OBS], F32, tag="nmean")
            nc.vector.tensor_scalar_mul(nmean, mean_t, -1.0)
            rstd = spool.tile([P, OBS], F32, tag="rstd")
            nc.vector.reciprocal(rstd, std_t)
            x = xpool.tile([P, OBS], F32, tag="x", name=f"x_in_{g}")
            nc.vector.tensor_add(x, obs_t, nmean)
            nc.vector.tensor_mul(x, x, rstd)
            nc.vector.tensor_scalar(
                out=x, in0=x, scalar1=-5.0, scalar2=5.0,
                op0=ALU.max, op1=ALU.min,
            )

            # ---- MLP layers ----
            for li, (O, I) in enumerate(DIMS):
                bt = bpool.tile([P, O], F32, tag="bias", name=f"b_{g}_{li}")
                nc.sync.dma_start(out=bt, in_=b_d[li][n0 : n0 + P, :])
                y = xpool.tile([P, O], F32, tag="y", name=f"y_{g}_{li}")
                for c0 in range(0, O, OCHUNK[li]):
                    oc = min(OCHUNK[li], O - c0)
                    wt = wpool.tile([P, oc, I], F32, tag="w", name=f"w_{g}_{li}_{c0}")
                    nc.sync.dma_start(
                        out=wt, in_=W_d[li][n0 : n0 + P, c0 : c0 + oc, :]
                    )
                    if I <= 64:
                        # Layer 0: I is tiny, so per-o fused ops are
                        # overhead-dominated. Instead: one in-place batched
                        # multiply (x broadcast across the o dim via a
                        # step-0 AP) + one segmented 3D reduce.
                        x_b = bass.AP(
                            tensor=x.tensor,
                            offset=x.offset,
                            ap=[x.ap[0], [0, oc], x.ap[1]],
                        )
                        nc.vector.tensor_mul(wt, wt, x_b)
                        nc.vector.reduce_sum(
                            out=y[:, c0 : c0 + oc],
                            in_=wt,
                            axis=mybir.AxisListType.X,
                        )
                    else:
                        scr = spool.tile(
                            [P, I], F32, tag="scr", name=f"scr_{g}_{li}_{c0}"
                        )
                        for o in range(oc):
                            # accum_out = sum_i W[o,i]*x[i]  (custom DVE
                            # fused multiply-reduce; the ISA
                            # TENSOR_TENSOR_REDUCE opcode crashes TRN2
                            # hardware on this path)
                            nc.vector.affine_mul_reduce(
                                out=scr,
                                accum_out=y[:, c0 + o : c0 + o + 1],
                                in0=wt[:, o, :],
                                in1=x,
                                scale=1.0,
                                bias=0.0,
                            )
                nc.vector.tensor_add(y, y, bt)
                if li < len(DIMS) - 1:
                    # ELU(y) = exp(min(y,0)) + max(y,0) - 1
                    e = spool.tile([P, O], F32, tag="elu", name=f"e_{g}_{li}")
                    nc.vector.tensor_scalar_min(e, y, 0.0)
                    nc.scalar.activation(e, e, AF.Exp)
                    xn = xpool.tile([P, O], F32, tag="x", name=f"x_{g}_{li}")
                    nc.vector.scalar_tensor_tensor(
                        out=xn, in0=y, scalar=0.0, in1=e,
                        op0=ALU.max, op1=ALU.add,
                    )
                    nc.vector.tensor_scalar_add(xn, xn, -1.0)
                    x = xn
                else:
                    yt = xpool.tile([P, O], F32, tag="yt", name=f"yt_{g}")
                    nc.scalar.activation(yt, y, AF.Tanh)
                    nc.sync.dma_start(out=out_d[n0 : n0 + P, :], in_=yt)

    nc.compile()
    return nc


def _get_nc():
    if "nc" not in _cached:
        _cached["nc"] = _build_bass()
    return _cached["nc"]


def kernel(obs, mean, std, W0, b0, W1, b1, W2, b2, W3, b3, _trace=False):
    nc = _get_nc()
    full = {
        "obs": obs, "mean": mean, "std": std,
        "W0": W0, "b0": b0, "W1": W1, "b1": b1,
        "W2": W2, "b2": b2, "W3": W3, "b3": b3,
    }
    in_maps = []
    for c in range(N_CORES):
        sl = slice(c * NPC, (c + 1) * NPC)
        in_maps.append(
            {k: np.ascontiguousarray(np.asarray(v)[sl]) for k, v in full.items()}
        )
    res = run_bass_kernel_spmd(
        nc, in_maps, core_ids=list(range(N_CORES)), trace=_trace
    )
    _cached["last_results"] = res
    out = np.concatenate([res.results[c]["out"] for c in range(N_CORES)], axis=0)
    return out


# revision 10
# speedup vs baseline: 1.1435x; 1.0102x over previous
"""Trainium2 Bass kernel for nn_BatchedTeacherPolicy.

2048 independent per-teacher MLPs (obs-norm -> 48->512->256->128->12,
ELU between layers, tanh at the end). Pure data parallel: 256 teachers
per NeuronCore across 8 cores.

Layout: teacher-on-partition. Each SBUF partition holds one teacher's
weights/activations; the per-teacher matvec y[o] = b[o] + sum_i W[o,i]x[i]
is one fused DVE tensor_tensor_reduce per output neuron o, computed for
128 teachers (partitions) simultaneously. Weight DMAs are fully
contiguous per partition (W[n, o0:o1, :] blocks).
"""

from contextlib import ExitStack

import numpy as np

import concourse.bass as bass
import concourse.bacc as bacc
import concourse.tile as tile
from concourse import mybir
from concourse.bass_utils import run_bass_kernel_spmd

N, OBS = 2048, 48
DIMS = [(512, 48), (256, 512), (128, 256), (12, 128)]  # (out, in) per layer
N_CORES = 8
NPC = N // N_CORES  # teachers per core
P = 128             # partitions = teachers per group
G = NPC // P        # groups per core
# o-chunk per layer: sized so W DMA chunks are ~2-4 MB
OCHUNK = [128, 16, 32, 12]

F32 = mybir.dt.float32
AF = mybir.ActivationFunctionType
ALU = mybir.AluOpType

_cached = {}


def _build_bass():
    nc = bacc.Bacc(trn_type="TRN2", target_bir_lowering=False)

    obs_d = nc.dram_tensor("obs", [NPC, OBS], F32, kind="ExternalInput")
    mean_d = nc.dram_tensor("mean", [NPC, OBS], F32, kind="ExternalInput")
    std_d = nc.dram_tensor("std", [NPC, OBS], F32, kind="ExternalInput")
    W_d, b_d = [], []
    for li, (o, i) in enumerate(DIMS):
        W_d.append(nc.dram_tensor(f"W{li}", [NPC, o, i], F32, kind="ExternalInput"))
        b_d.append(nc.dram_tensor(f"b{li}", [NPC, o], F32, kind="ExternalInput"))
    out_d = nc.dram_tensor("out", [NPC, DIMS[-1][0]], F32, kind="ExternalOutput")

    with ExitStack() as ctx:
        tc = ctx.enter_context(tile.TileContext(nc))
        wpool = ctx.enter_context(tc.tile_pool(name="wpool", bufs=5))
        xpool = ctx.enter_context(tc.tile_pool(name="xpool", bufs=3))
        spool = ctx.enter_context(tc.tile_pool(name="spool", bufs=2))
        bpool = ctx.enter_context(tc.tile_pool(name="bpool", bufs=2))

        for g in range(G):
            n0 = g * P

            # ---- obs normalization: x0 = clip((obs - mean)/std, -5, 5) ----
            obs_t = spool.tile([P, OBS], F32, tag="nrm")
            nc.sync.dma_start(out=obs_t, in_=obs_d[n0 : n0 + P, :])
            mean_t = spool.tile([P, OBS], F32, tag="nrm")
            nc.sync.dma_start(out=mean_t, in_=mean_d[n0 : n0 + P, :])
            std_t = spool.tile([P, OBS], F32, tag="nrm")
            nc.sync.dma_start(out=std_t, in_=std_d[n0 : n0 + P, :])

            # Each DVE op may carry at most ONE new semaphore wait (TRN2
            # TT-struct limit), so feed multi-operand ops through
            # single-input ops that absorb the DMA waits first.
            nmean = spool.tile([P, OBS], F32, tag="nmean")
            nc.vector.tensor_scalar_mul(nmean, mean_t, -1.0)
            rstd = spool.tile([P, OBS], F32, tag="rstd")
            nc.vector.reciprocal(rstd, std_t)
            x = xpool.tile([P, OBS], F32, tag="x", name=f"x_in_{g}")
            nc.vector.tensor_add(x, obs_t, nmean)
            nc.vector.tensor_mul(x, x, rstd)
            nc.vector.tensor_scalar(
                out=x, in0=x, scalar1=-5.0, scalar2=5.0,
                op0=ALU.max, op1=ALU.min,
            )

            # ---- MLP layers ----
            for li, (O, I) in enumerate(DIMS):
                bt = bpool.tile([P, O], F32, tag="bias", name=f"b_{g}_{li}")
                nc.sync.dma_start(out=bt, in_=b_d[li][n0 : n0 + P, :])
                y = xpool.tile([P, O], F32, tag="y", name=f"y_{g}_{li}")
                for c0 in range(0, O, OCHUNK[li]):
                    oc = min(OCHUNK[li], O - c0)
                    wt = wpool.tile([P, oc, I], F32, tag="w", name=f"w_{g}_{li}_{c0}")
                    nc.sync.dma_start(
                        out=wt, in_=W_d[li][n0 : n0 + P, c0 : c0 + oc, :]
                    )
                    if I <= 64:
                        # Layer 0: I is tiny, so per-o fused ops are
                        # overhead-dominated. Instead: one in-place batched
                        # multiply (x broadcast across the o dim via a
                        # step-0 AP) + one segmented 3D reduce.
                        x_b = bass.AP(
                            tensor=x.tensor,
                            offset=x.offset,
                            ap=[x.ap[0], [0, oc], x.ap[1]],
                        )
                        nc.vector.tensor_mul(wt, wt, x_b)
                        nc.vector.reduce_sum(
                            out=y[:, c0 : c0 + oc],
                            in_=wt,
                            axis=mybir.AxisListType.X,
                        )
                    else:
                        scr = spool.tile(
                            [P, I], F32, tag="scr", name=f"scr_{g}_{li}_{c0}"
                        )
                        for o in range(oc):
                            # accum_out = sum_i W[o,i]*x[i]  (custom DVE
                            # fused multiply-reduce; the ISA
                            # TENSOR_TENSOR_REDUCE opcode crashes TRN2
                            # hardware on this path)
                            nc.vector.affine_mul_reduce(
                                out=scr,
                                accum_out=y[:, c0 + o : c0 + o + 1],
                                in0=wt[:, o, :],
                                in1=x,
                                scale=1.0,
                                bias=0.0,
                            )
                nc.vector.tensor_add(y, y, bt)
                if li < len(DIMS) - 1:
                    # ELU(y) = exp(min(y,0)) + max(y,0) - 1
                    e = spool.tile([P, O], F32, tag="elu", name=f"e_{g}_{li}")
                    nc.vector.tensor_scalar_min(e, y, 0.0)
                    nc.scalar.activation(e, e, AF.Exp)
                    xn = xpool.tile([P, O], F32, tag="x", name=f"x_{g}_{li}")
                    nc.vector.scalar_tensor_tensor(
                        out=xn, in0=y, scalar=0.0, in1=e,
                        op0=ALU.max, op1=ALU.add,
                    )
                    nc.vector.tensor_scalar_add(xn, xn, -1.0)
                    x = xn
                else:
                    yt = xpool.tile([P, O], F32, tag="yt", name=f"yt_{g}")
                    nc.scalar.activation(yt, y, AF.Tanh)
                    nc.sync.dma_start(out=out_d[n0 : n0 + P, :], in_=yt)

    nc.compile()
    return nc


def _get_nc():
    if "nc" not in _cached:
        _cached["nc"] = _build_bass()
    return _cached["nc"]


def kernel(obs, mean, std, W0, b0, W1, b1, W2, b2, W3, b3, _trace=False):
    nc = _get_nc()
    full = {
        "obs": obs, "mean": mean, "std": std,
        "W0": W0, "b0": b0, "W1": W1, "b1": b1,
        "W2": W2, "b2": b2, "W3": W3, "b3": b3,
    }
    in_maps = []
    for c in range(N_CORES):
        sl = slice(c * NPC, (c + 1) * NPC)
        in_maps.append(
            {k: np.ascontiguousarray(np.asarray(v)[sl]) for k, v in full.items()}
        )
    res = run_bass_kernel_spmd(
        nc, in_maps, core_ids=list(range(N_CORES)), trace=_trace
    )
    _cached["last_results"] = res
    out = np.concatenate([res.results[c]["out"] for c in range(N_CORES)], axis=0)
    return out
